# revision 40
# baseline (speedup 1.0000x reference)
"""Trainium2 Bass kernel for GNN message passing (nn_Kernel_17772574670927).

Reference computes, per node b with N=8 neighbors:
    out[b] = sum_n concat(node_v[b], node_h[b], nbr_v[b,n], nbr_h[b,n]) @ W + bias
Since the linear layer distributes over the neighbor sum:
    out[b] = (N*node_v[b])*W[0] + (N*node_h[b]) @ W[1:257]
           + (sum_n nbr_v[b,n])*W[257] + (sum_n nbr_h[b,n]) @ W[258:514] + N*bias
an 8x FLOP reduction vs the naive form; the kernel is then HBM-bound on
streaming nbr_h (512 MB total).

Per 128-node tile: the 8-neighbor sum is a 3-level contiguous add tree on
DVE (fp32, last level emits bf16); node_h/nbr-sum blocks are PE-transposed
to feature-major and the 2*(1+H) x H GEMM runs in bf16 (tolerance is 2e-2),
with the v-features and bias folded into a single K=3 matmul whose lhsT
columns (node_v, nbr_v sum, ones) are precomputed for all tiles.

Sharding: data-parallel over the node dim B=65536 across 8 cores (8192 each).
W/bias replicated. No collectives.
"""
import sys

for _p in ("/root/.axon_site", "/root/.axon_site/_ro/trn_rl_repo", "/opt/trn_rl_repo"):
    if _p not in sys.path:
        sys.path.append(_p)

import numpy as np

import concourse.bacc as bacc
import concourse.bass as bass
import concourse.mybir as mybir
from concourse.masks import make_identity
from concourse.tile import TileContext

B, N, H = 65536, 8, 256
NCORES = 8
BP = B // NCORES          # 8192 nodes per core
P = 128                   # SBUF partitions
NTILES = BP // P          # 64 node-tiles per core
CHUNK = 8                 # tiles per node_h/out DMA chunk (1 MiB transfers)
NCHUNKS = NTILES // CHUNK
F32 = mybir.dt.float32
BF16 = mybir.dt.bfloat16

def build_bass() -> bass.Bass:
    nc = bacc.Bacc("TRN2", target_bir_lowering=False, debug=False,
                   num_devices=NCORES)
    node_v = nc.dram_tensor("node_v", [BP, 1], F32, kind="ExternalInput")
    node_h = nc.dram_tensor("node_h", [BP, H], F32, kind="ExternalInput")
    nbr_v = nc.dram_tensor("nbr_v", [BP, N, 1], F32, kind="ExternalInput")
    nbr_h = nc.dram_tensor("nbr_h", [BP, N, H], F32, kind="ExternalInput")
    W = nc.dram_tensor("W", [2 * (1 + H), H], F32, kind="ExternalInput")
    bvec = nc.dram_tensor("b", [H], F32, kind="ExternalInput")
    # bf16 output halves the write traffic; host upcasts to fp32 after gather
    out = nc.dram_tensor("out", [BP, H], BF16, kind="ExternalOutput")

    with TileContext(nc) as tc, nc.allow_low_precision(
        reason="bf16 GEMM inputs; harness tolerance is 2e-2"
    ):
        with (
            tc.tile_pool(name="singles", bufs=1) as singles,
            tc.tile_pool(name="nbr", bufs=8) as nbr_pool,
            tc.tile_pool(name="a1", bufs=3) as a1_pool,
            tc.tile_pool(name="a2", bufs=3) as a2_pool,
            tc.tile_pool(name="nbrsum", bufs=3) as nbrsum_pool,
            tc.tile_pool(name="nodeh", bufs=3) as nodeh_pool,
            tc.tile_pool(name="outp", bufs=3) as out_pool,
            tc.tile_pool(name="xt", bufs=4) as xt_pool,
            tc.tile_pool(name="vsmall", bufs=3) as vsmall_pool,
            tc.tile_pool(name="ptf", bufs=2, space="PSUM") as ptf_pool,
            tc.tile_pool(name="ptb", bufs=2, space="PSUM") as ptb_pool,
            tc.tile_pool(name="pvpack", bufs=1, space="PSUM") as vpackT_pool,
            tc.tile_pool(name="pout", bufs=2, space="PSUM") as psum_out_pool,
        ):
            # ---- pre-issue the first reads so HBM streams from t=0 ----
            # (the sync sequencer issues DMAs in program order; putting these
            # ahead of the setup chain removes the startup bubble)
            pre_nbr = []
            for t0 in range(8):
                pn = nbr_pool.tile([P, N, H], F32, tag="nbr")
                nc.sync.dma_start(out=pn[:], in_=nbr_h[t0 * P:(t0 + 1) * P, :, :])
                pre_nbr.append(pn)
            pre_nodeh = []
            for c0 in range(2):
                ph = nodeh_pool.tile([P, CHUNK, H], F32, tag="nodeh")
                nc.scalar.dma_start(
                    out=ph[:],
                    in_=node_h[c0 * CHUNK * P:(c0 + 1) * CHUNK * P, :].rearrange(
                        "(t p) h -> p t h", p=P
                    ),
                )
                pre_nodeh.append(ph)

            # ---- one-time setup ----
            identity = singles.tile([P, P], F32)
            make_identity(nc, identity)
            identity_bf = singles.tile([P, P], BF16)
            nc.scalar.copy(out=identity_bf[:], in_=identity[:])

            # W chunks for the h-features: rows 1:257 (self) and 258:514 (nbr)
            w_f32 = singles.tile([P, 4, H], F32)
            nc.sync.dma_start(out=w_f32[:, 0, :], in_=W[1:129, :])
            nc.sync.dma_start(out=w_f32[:, 1, :], in_=W[129:257, :])
            nc.sync.dma_start(out=w_f32[:, 2, :], in_=W[258:386, :])
            nc.sync.dma_start(out=w_f32[:, 3, :], in_=W[386:514, :])
            # self-feature weights absorb the xN from the neighbor sum
            nc.scalar.mul(w_f32[:, 0, :], w_f32[:, 0, :], float(N))
            nc.scalar.mul(w_f32[:, 1, :], w_f32[:, 1, :], float(N))
            w_sb = singles.tile([P, 4, H], BF16)
            nc.scalar.copy(out=w_sb[:], in_=w_f32[:])

            # v-feature + bias weights as one K=3 rhs:
            #   row0 = W[0] (self v), row1 = W[257] (nbr v), row2 = b
            # (the xN for self-v and bias lives in the lhsT columns instead)
            vw_f32 = singles.tile([3, H], F32)
            nc.sync.dma_start(out=vw_f32[0:1, :], in_=W[0:1, :])
            nc.sync.dma_start(out=vw_f32[1:2, :], in_=W[257:258, :])
            nc.sync.dma_start(out=vw_f32[2:3, :], in_=bvec[:])
            v_w3 = singles.tile([3, H], BF16)
            nc.scalar.copy(out=v_w3[:], in_=vw_f32[:])

            # node_v rows: nodev_sb[t, p] = N * node_v[t*128 + p]
            nodev_sb = singles.tile([NTILES, P], F32)
            nc.sync.dma_start(
                out=nodev_sb[:], in_=node_v.rearrange("(t p) o -> t (p o)", p=P)
            )
            nc.scalar.mul(nodev_sb[:], nodev_sb[:], float(N))
            # nbr_v rows: [t, p, n]
            nbrv_raw = singles.tile([NTILES, P, N], F32)
            nc.sync.dma_start(
                out=nbrv_raw[:], in_=nbr_v.rearrange("(t p) n o -> t p (n o)", p=P)
            )
            nbrvsum_sb = singles.tile([NTILES, P], F32)
            nc.vector.tensor_reduce(
                out=nbrvsum_sb[:], in_=nbrv_raw[:],
                axis=mybir.AxisListType.X, op=mybir.AluOpType.add,
            )
            # transpose both to column form [128, NTILES], pack with a ones
            # column into the per-tile K=3 lhsT source [128, t, 3]
            vcols_f32 = singles.tile([P, NTILES, 3], F32)
            pv0 = ptf_pool.tile([P, P], F32, tag="ptf")
            nc.tensor.transpose(
                pv0[:, :NTILES], nodev_sb[:], identity[:NTILES, :NTILES]
            )
            nc.vector.tensor_copy(out=vcols_f32[:, :, 0], in_=pv0[:, :NTILES])
            pv1 = ptf_pool.tile([P, P], F32, tag="ptf")
            nc.tensor.transpose(
                pv1[:, :NTILES], nbrvsum_sb[:], identity[:NTILES, :NTILES]
            )
            nc.vector.tensor_copy(out=vcols_f32[:, :, 1], in_=pv1[:, :NTILES])
            nc.gpsimd.memset(vcols_f32[:, :, 2], float(N))
            vcols_all = singles.tile([P, NTILES, 3], BF16)
            nc.scalar.copy(out=vcols_all[:], in_=vcols_f32[:])

            # ---- main loop ----
            for c in range(NCHUNKS):
                lo = c * CHUNK * P
                hi = (c + 1) * CHUNK * P
                if c < 2:
                    nodeh_chunk = pre_nodeh[c]
                else:
                    nodeh_chunk = nodeh_pool.tile([P, CHUNK, H], F32, tag="nodeh")
                    # node_h reads ride the Act queue, off the nbr stream
                    nc.scalar.dma_start(
                        out=nodeh_chunk[:],
                        in_=node_h[lo:hi, :].rearrange("(t p) h -> p t h", p=P),
                    )

                for j in range(CHUNK):
                    t = c * CHUNK + j
                    if t < 8:
                        nbr_tile = pre_nbr[t]
                    else:
                        nbr_tile = nbr_pool.tile([P, N, H], F32, tag="nbr")
                        nc.sync.dma_start(
                            out=nbr_tile[:], in_=nbr_h[t * P:(t + 1) * P, :, :]
                        )
                    # neighbor sum: 3-level contiguous add tree on DVE
                    a1 = a1_pool.tile([P, 4, H], F32)
                    nc.vector.tensor_add(
                        out=a1[:], in0=nbr_tile[:, 0:4, :], in1=nbr_tile[:, 4:8, :]
                    )
                    a2 = a2_pool.tile([P, 2, H], F32)
                    nc.vector.tensor_add(
                        out=a2[:], in0=a1[:, 0:2, :], in1=a1[:, 2:4, :]
                    )

                    # transpose the four 128-feature blocks to feature-major
                    xt = xt_pool.tile([P, 4, P], BF16)
                    for i, src in enumerate(
                        (nodeh_chunk[:, j, 0:128], nodeh_chunk[:, j, 128:256])
                    ):
                        ptf = ptf_pool.tile([P, P], F32, tag="ptf")
                        nc.tensor.transpose(ptf[:], src, identity[:])
                        nc.scalar.copy(out=xt[:, i, :], in_=ptf[:])
                    # final neighbor-add folded into PSUM-accumulating
                    # transposes: xt_nbr = T(a2[:,0,blk]) + T(a2[:,1,blk])
                    for i in range(2):
                        ptb = ptb_pool.tile([P, P], F32, tag="ptb")
                        nc.tensor.matmul(
                            ptb[:], a2[:, 0, i * 128:(i + 1) * 128],
                            identity[:], start=True, stop=False,
                            is_transpose=True,
                        )
                        nc.tensor.matmul(
                            ptb[:], a2[:, 1, i * 128:(i + 1) * 128],
                            identity[:], start=False, stop=True,
                            is_transpose=True,
                        )
                        nc.scalar.copy(out=xt[:, 2 + i, :], in_=ptb[:])

                    # K=3 v/bias lhsT: [128, 3] column triple -> [3, 128]
                    vpT = vpackT_pool.tile([3, P], BF16)
                    nc.tensor.transpose(vpT[:], vcols_all[:, t, :], identity_bf[:])
                    vpack = vsmall_pool.tile([3, P], BF16, tag="vpack")
                    nc.scalar.copy(out=vpack[:], in_=vpT[:])

                    # accumulate all five K-blocks into PSUM (bias included)
                    psum_out = psum_out_pool.tile([P, H], F32)
                    for i in range(4):
                        nc.tensor.matmul(
                            psum_out[:], xt[:, i, :], w_sb[:, i, :],
                            start=(i == 0), stop=False,
                        )
                    nc.tensor.matmul(
                        psum_out[:], vpack[:], v_w3[:], start=False, stop=True
                    )

                    # per-tile out write: lands while later tiles still
                    # compute, so the big nbr reads never queue behind it
                    out_tile = out_pool.tile([P, H], BF16)
                    nc.scalar.copy(out=out_tile[:], in_=psum_out[:])
                    # write via the GpSimd software DGE: its own queue, so the
                    # data-dependent writes never head-of-line-block the
                    # always-ready reads on the SP/Act HWDGE queues
                    nc.gpsimd.dma_start(
                        out=out[t * P:(t + 1) * P, :], in_=out_tile[:]
                    )
    nc.compile()
    return nc


_BASS_CACHE = None


def _get_bass():
    global _BASS_CACHE
    if _BASS_CACHE is None:
        _BASS_CACHE = build_bass()
    return _BASS_CACHE


def run_sharded(inputs: dict, trace: bool = False, trace_cores=None):
    """Shard full inputs over 8 cores, run, gather. Returns (out, results)."""
    from concourse.bass_utils import run_bass_kernel_spmd

    nc = _get_bass()
    node_v = np.ascontiguousarray(np.asarray(inputs["node_v"], dtype=np.float32))
    node_h = np.ascontiguousarray(np.asarray(inputs["node_h"], dtype=np.float32))
    nbr_v = np.ascontiguousarray(np.asarray(inputs["nbr_v"], dtype=np.float32))
    nbr_h = np.ascontiguousarray(np.asarray(inputs["nbr_h"], dtype=np.float32))
    Wf = np.ascontiguousarray(np.asarray(inputs["W"], dtype=np.float32))
    bf = np.ascontiguousarray(np.asarray(inputs["b"], dtype=np.float32))

    in_maps = []
    for core in range(NCORES):
        s = slice(core * BP, (core + 1) * BP)
        in_maps.append({
            "node_v": node_v[s], "node_h": node_h[s],
            "nbr_v": nbr_v[s], "nbr_h": nbr_h[s],
            "W": Wf, "b": bf,
        })
    kwargs = {}
    if trace:
        kwargs.update(trace=True, trace_cores=trace_cores or [0])
    res = run_bass_kernel_spmd(nc, in_maps, core_ids=list(range(NCORES)), **kwargs)
    full = np.concatenate(
        [np.asarray(res.results[i]["out"]).astype(np.float32) for i in range(NCORES)],
        axis=0,
    )
    return full, res


def kernel(**inputs) -> np.ndarray:
    out, _ = run_sharded(inputs, trace=False)
    return out


if __name__ == "__main__":
    rng = np.random.default_rng(0)
    fake = {
        "node_v": rng.standard_normal((B, 1), dtype=np.float32),
        "node_h": rng.standard_normal((B, H), dtype=np.float32),
        "nbr_v": rng.standard_normal((B, N, 1), dtype=np.float32),
        "nbr_h": rng.standard_normal((B, N, H), dtype=np.float32),
        "W": rng.standard_normal((514, H), dtype=np.float32) / np.sqrt(514),
        "b": np.zeros((H,), dtype=np.float32),
        "iteration": 0,
    }
    got = kernel(**fake)
    sf = np.concatenate([fake["node_v"], fake["node_h"]], axis=-1)
    nf = np.concatenate([fake["nbr_v"], fake["nbr_h"]], axis=-1)
    exp = (
        N * sf @ fake["W"][:257] + nf.sum(axis=1) @ fake["W"][257:] + N * fake["b"]
    )
    err = np.abs(got - exp).max() / np.abs(exp).max()
    print("rel err vs numpy:", err)
